# revision 29
# baseline (speedup 1.0000x reference)
"""Trainium2 Bass kernel for an LSTM caption decoder (DecoderRNN).

Math (fp32 reference):
  x_seq = [features; embedding[captions[:, :-1]]]      # [B, T, E]
  xg    = x_seq @ W + b                                # [T*B, 4H] (t-major)
  per step t: gates = xg_t + h @ U ; i,f,o=sigmoid, g=tanh
              c = f*c + i*g ; h = o*tanh(c)
  logits = hs @ linear_w.T + linear_b                  # [B, T, V]

B=64, T=64, E=512, H=1024, V=32000.

Distribution over 8 NeuronCores: the serial recurrence (and the xg GEMM
feeding it) is replicated on every core — its PE cost is K*N-bound and
independent of batch, so batch-splitting buys nothing and collectives
would cost more than the redundant compute. The vocab projection (84% of
total FLOPs) is sharded column-wise: core c computes logits[:, :, c*4000:
(c+1)*4000]. No collectives; the host concatenates the vocab slices.

All matmuls run in float32r (TF32: full PE rate, fp32 accumulate).
Weights/activations feeding matmuls are pre-rounded to the TF32 grid.
"""

from contextlib import ExitStack

import numpy as np

import concourse.bass as bass
import concourse.mybir as mybir
import concourse.tile as tile
from concourse import bacc, bass_utils
from concourse.bass import ds

F32 = mybir.dt.float32
F32R = mybir.dt.float32r
ACTF = mybir.ActivationFunctionType
import os as _os
IMM = _os.environ.get("KIMM", "1") == "1"

B, T, E, H, V = 64, 64, 512, 1024, 32000
NCORES = 8
VS = V // NCORES          # vocab slice per core (4000)
G4 = 4 * H                # 4096
TOK = T * B               # 4096 tokens, t-major (row = t*B + b)
EC = E // 128             # 4  E chunks
HC = H // 128             # 8  H chunks
MC = TOK // 128           # 32 token chunks (2 timesteps each)
NV = 8                    # vocab N-chunks per core
VN = VS // NV             # 500 columns per vocab N-chunk


def _round_tf32(x: np.ndarray) -> np.ndarray:
    """Round fp32 to the TF32 grid (RNE to 10 mantissa bits)."""
    bits = np.ascontiguousarray(x, dtype=np.float32).view(np.uint32)
    r = bits + np.uint32(0xFFF) + ((bits >> np.uint32(13)) & np.uint32(1))
    return (r & np.uint32(0xFFFFE000)).view(np.float32)


def _emit(ctx: ExitStack, tc: tile.TileContext, io: dict, phases="abc"):
    nc = tc.nc
    xT_d, w_d, u_d, bg_d, ident_d, wt_d, bl_d = (
        io["xT"], io["w"], io["u"], io["bg"], io["ident"], io["wt"], io["bl"])
    xg_d, hsT_d, out_d = io["xg_scratch"], io["hsT_scratch"], io["logits"]

    if "a" in phases:
        _phase_a(tc, nc, xT_d, w_d, bg_d, xg_d)
    if "b" in phases:
        _phase_b(tc, nc, u_d, ident_d, xg_d, hsT_d)
    if "c" in phases:
        _phase_c(tc, nc, wt_d, bl_d, hsT_d, out_d)


def _phase_a(tc, nc, xT_d, w_d, bg_d, xg_d):
    # ---------------- Phase A: xg = x @ W + b  ->  DRAM [TOK, 4H] --------
    with tc.tile_pool(name="a_sb", bufs=1) as a_sb, \
         tc.tile_pool(name="a_out", bufs=3) as a_out, \
         tc.tile_pool(name="a_ps", bufs=2, space="PSUM") as a_ps:
        xT_sb = [a_sb.tile([128, TOK], F32R, tag=f"xT{k}", name=f"xT{k}") for k in range(EC)]
        w_sb = [a_sb.tile([128, G4], F32R, tag=f"w{k}", name=f"wsb{k}") for k in range(EC)]
        bg_sb = a_sb.tile([128, G4], F32, tag="bg")
        for k in range(EC):
            nc.sync.dma_start(xT_sb[k][:], xT_d[k * 128:(k + 1) * 128, :])
            nc.sync.dma_start(w_sb[k][:], w_d[k * 128:(k + 1) * 128, :])
        nc.sync.dma_start(bg_sb[:], bg_d[:, :])

        for m in range(MC):
            for half in range(2):
                ps = a_ps.tile([128, 2048], F32)
                for n in range(4):
                    col = half * 2048 + n * 512
                    for k in range(EC):
                        nc.tensor.matmul(
                            ps[:, n * 512:(n + 1) * 512],
                            xT_sb[k][:, m * 128:(m + 1) * 128],
                            w_sb[k][:, col:col + 512],
                            start=(k == 0), stop=(k == EC - 1))
                o_sb = a_out.tile([128, 2048], F32R)
                nc.vector.tensor_add(
                    o_sb[:], ps[:], bg_sb[:, half * 2048:(half + 1) * 2048])
                nc.sync.dma_start(
                    xg_d[m * 128:(m + 1) * 128, half * 2048:(half + 1) * 2048],
                    o_sb[:])

def _phase_b(tc, nc, u_d, ident_d, xg_d, hsT_d):
    # ---------------- Phase B: the recurrence ---------------------------
    # gate column ranges in xg/U: i=[0,H), f=[H,2H), g=[2H,3H), o=[3H,4H)
    # PE order i, g, f, o so the c-chain (needs i,g,f) starts earliest.
    # xg_t is injected into PSUM via an identity matmul (keeps the DVE off
    # the critical path); each gate gets its own 2-bank PSUM tile.
    with tc.tile_pool(name="b_sb", bufs=1) as b_sb, \
         tc.tile_pool(name="b_xg", bufs=7) as b_xg, \
         tc.tile_pool(name="b_gate", bufs=6) as b_gate, \
         tc.tile_pool(name="b_tmp", bufs=3) as b_tmp, \
         tc.tile_pool(name="b_hT", bufs=24) as b_hT, \
         tc.tile_pool(name="b_ps", bufs=4, space="PSUM") as b_ps:
        u_sb = [b_sb.tile([128, G4], F32R, tag=f"u{k}", name=f"usb{k}") for k in range(HC)]
        for k in range(HC):
            nc.sync.dma_start(u_sb[k][:], u_d[k * 128:(k + 1) * 128, :])
        ident = b_sb.tile([64, 64], F32R, tag="ident")
        nc.sync.dma_start(ident[:], ident_d[:, :])
        c_st = b_sb.tile([64, H], F32, tag="c")     # persistent cell state
        h_st = b_sb.tile([64, H], F32, tag="h")     # persistent hidden (pre-T)

        hT_prev = None
        for t in range(T):
            # xg_t gate slices [64, H] each (f32r, bias already folded in)
            xg_g = []
            for g in range(4):
                xt = b_xg.tile([64, H], F32R, tag="xg")
                nc.sync.dma_start(
                    xt[:], xg_d[t * 64:(t + 1) * 64, g * H:(g + 1) * H])
                xg_g.append(xt)

            def gate_psum(g):
                """MMs for gate g: psum = I.T @ xg_g + sum_k hT_k.T @ U_k."""
                ps = b_ps.tile([64, H], F32, tag="ps", name=f"ps{g}_{t}")
                for n2 in range(2):
                    sl = ps[:, n2 * 512:(n2 + 1) * 512]
                    if IMM or t == 0:
                        nc.tensor.matmul(
                            sl, ident[:], xg_g[g][:, n2 * 512:(n2 + 1) * 512],
                            start=True, stop=(t == 0))
                    if t > 0:
                        ucol = g * H + n2 * 512
                        for k in range(HC):
                            nc.tensor.matmul(
                                sl, hT_prev[k][:],
                                u_sb[k][:, ucol:ucol + 512],
                                start=(not IMM and k == 0),
                                stop=(k == HC - 1))
                return ps

            def act(g, ps):
                a = b_gate.tile([64, H], F32, tag="gate", name=f"gate{g}_{t}")
                func = ACTF.Tanh if g == 2 else ACTF.Sigmoid
                for n2 in range(2):
                    sl = slice(n2 * 512, (n2 + 1) * 512)
                    if not IMM and t > 0:
                        nc.vector.tensor_add(ps[:, sl], ps[:, sl],
                                             xg_g[g][:, sl])
                    nc.scalar.activation(a[:, sl], ps[:, sl], func)
                return a

            # i, g first -> DVE ig overlaps f/o matmuls
            ps_i = gate_psum(0)
            i_t = act(0, ps_i)
            ps_g = gate_psum(2)
            g_t = act(2, ps_g)
            ig = b_tmp.tile([64, H], F32, tag="tmp", name=f"ig{t}")
            nc.vector.tensor_mul(ig[:], i_t[:], g_t[:])
            ps_f = gate_psum(1)
            f_t = act(1, ps_f)
            if t == 0:
                nc.vector.tensor_copy(c_st[:], ig[:])
            else:
                nc.vector.tensor_mul(c_st[:], f_t[:], c_st[:])
                nc.vector.tensor_add(c_st[:], c_st[:], ig[:])
            tc_t = b_tmp.tile([64, H], F32, tag="tmp", name=f"tc{t}")
            nc.scalar.activation(tc_t[:], c_st[:], ACTF.Tanh)
            ps_o = gate_psum(3)
            o_t = act(3, ps_o)
            # h in two halves so transposes/next-step MMs start earlier
            for hh in range(2):
                sl = slice(hh * 512, (hh + 1) * 512)
                nc.vector.tensor_mul(h_st[:, sl], o_t[:, sl], tc_t[:, sl])

            # transpose h [64, H] -> hT [H, 64] chunks, cast to f32r.
            # One tile per chunk so next-step MM k only waits on copy k.
            hT = []
            for k in range(HC):
                tp = b_ps.tile([128, 64], F32, tag="ps")
                nc.tensor.transpose(tp[:], h_st[:, k * 128:(k + 1) * 128],
                                    ident[:].bitcast(F32))
                ht_k = b_hT.tile([128, 64], F32R, tag="hT", name=f"hT{k}_{t}")
                nc.vector.tensor_copy(ht_k[:], tp[:])
                nc.scalar.dma_start(
                    hsT_d[:, t * 512 + k * 64:t * 512 + (k + 1) * 64], ht_k[:])
                hT.append(ht_k)
            hT_prev = hT

def _phase_c(tc, nc, wt_d, bl_d, hsT_d, out_d):
    # ---------------- Phase C: logits slice = hs @ WT + bl ---------------
    with tc.tile_pool(name="c_sb", bufs=1) as c_sb, \
         tc.tile_pool(name="c_hs", bufs=24) as c_hs, \
         tc.tile_pool(name="c_out", bufs=12) as c_out, \
         tc.tile_pool(name="c_ps", bufs=8, space="PSUM") as c_ps:
        wt_sb = [c_sb.tile([128, VS], F32R, tag=f"wt{k}", name=f"wtsb{k}") for k in range(HC)]
        for k in range(HC):
            nc.sync.dma_start(wt_sb[k][:], wt_d[k * 128:(k + 1) * 128, :])
        bl_sb = c_sb.tile([128, VS], F32, tag="bl")
        nc.sync.dma_start(bl_sb[:], bl_d[:, :])

        hsT4 = hsT_d.rearrange("p (t k b) -> p t k b", t=T, k=HC, b=64)
        for m in range(MC):
            t0, t1 = 2 * m, 2 * m + 1
            hs_k = []
            for k in range(HC):
                hk = c_hs.tile([128, 128], F32R, tag="slab", name=f"hk{k}_{m}")
                nc.sync.dma_start(
                    hk[:].rearrange("p (t b) -> p t b", t=2, b=64),
                    hsT4[:, t0:t0 + 2, k, :])
                hs_k.append(hk)
            for n in range(NV):
                ps = c_ps.tile([128, VN], F32)
                for k in range(HC):
                    nc.tensor.matmul(
                        ps[:], hs_k[k][:],
                        wt_sb[k][:, n * VN:(n + 1) * VN],
                        start=(k == 0), stop=(k == HC - 1))
                o_sb = c_out.tile([128, VN], F32)
                nc.vector.tensor_add(o_sb[:], ps[:],
                                     bl_sb[:, n * VN:(n + 1) * VN])
                eng0 = (nc.sync, nc.scalar)[n % 2]
                eng1 = (nc.scalar, nc.sync)[n % 2]
                eng0.dma_start(out_d[:, t0, ds(n * VN, VN)], o_sb[0:64, :])
                eng1.dma_start(out_d[:, t1, ds(n * VN, VN)], o_sb[64:128, :])


def build_program(phases=None):
    import os
    if phases is None:
        phases = os.environ.get("KPHASES", "abc")
    nc = bacc.Bacc("TRN2", target_bir_lowering=False, debug=False,
                   num_devices=NCORES)
    io = {
        "xT": nc.dram_tensor("xT", [E, TOK], F32R, kind="ExternalInput").ap(),
        "w": nc.dram_tensor("w", [E, G4], F32R, kind="ExternalInput").ap(),
        "u": nc.dram_tensor("u", [H, G4], F32R, kind="ExternalInput").ap(),
        "bg": nc.dram_tensor("bg", [128, G4], F32, kind="ExternalInput").ap(),
        "ident": nc.dram_tensor("ident", [64, 64], F32R,
                                kind="ExternalInput").ap(),
        "wt": nc.dram_tensor("wt", [H, VS], F32R, kind="ExternalInput").ap(),
        "bl": nc.dram_tensor("bl", [128, VS], F32, kind="ExternalInput").ap(),
        "xg_scratch": nc.dram_tensor("xg_scratch", [TOK, G4], F32R,
                                     kind="Internal").ap(),
        "hsT_scratch": nc.dram_tensor("hsT_scratch", [128, T * 512], F32R,
                                      kind="Internal").ap(),
        "logits": nc.dram_tensor("logits", [B, T, VS], F32,
                                 kind="ExternalOutput").ap(),
    }
    with tile.TileContext(nc) as tc:
        with ExitStack() as ctx:
            _emit(ctx, tc, io, phases)
    nc.compile()
    return nc


def make_in_maps(features, captions, embedding, W_i, U_i, b_i, W_f, U_f, b_f,
                 W_g, U_g, b_g, W_o, U_o, b_o, linear_w, linear_b):
    features = np.asarray(features, dtype=np.float32)
    captions = np.asarray(captions)
    embedding = np.asarray(embedding, dtype=np.float32)
    emb = embedding[captions[:, :-1]]                        # [B, T-1, E]
    x_seq = np.concatenate([features[:, None, :], emb], axis=1)  # [B, T, E]
    x_flat = np.ascontiguousarray(
        x_seq.transpose(1, 0, 2).reshape(TOK, E))            # t-major tokens
    xT = _round_tf32(np.ascontiguousarray(x_flat.T))         # [E, TOK]

    w = _round_tf32(np.concatenate([W_i, W_f, W_g, W_o], axis=1))  # [E, 4H]
    u = _round_tf32(np.concatenate([U_i, U_f, U_g, U_o], axis=1))  # [H, 4H]
    bgv = np.concatenate([b_i, b_f, b_g, b_o], axis=0).astype(np.float32)
    bg = np.ascontiguousarray(np.broadcast_to(bgv[None, :], (128, G4)))
    ident = np.eye(64, dtype=np.float32)

    linear_w = np.asarray(linear_w, dtype=np.float32)
    linear_b = np.asarray(linear_b, dtype=np.float32)
    common = {"xT": xT, "w": w, "u": u, "bg": bg, "ident": ident}
    in_maps = []
    for c in range(NCORES):
        wt = _round_tf32(
            np.ascontiguousarray(linear_w[c * VS:(c + 1) * VS, :].T))
        bl = np.ascontiguousarray(np.broadcast_to(
            linear_b[None, c * VS:(c + 1) * VS], (128, VS)))
        in_maps.append({**common, "wt": wt, "bl": bl})
    return in_maps


_PROGRAM = None


def kernel(**inputs) -> np.ndarray:
    global _PROGRAM
    if _PROGRAM is None:
        _PROGRAM = build_program()
    in_maps = make_in_maps(**inputs)
    res = bass_utils.run_bass_kernel_spmd(
        _PROGRAM, in_maps, core_ids=list(range(NCORES)))
    out = np.empty((B, T, V), dtype=np.float32)
    for c in range(NCORES):
        out[:, :, c * VS:(c + 1) * VS] = res.results[c]["logits"]
    return out


# revision 43
# speedup vs baseline: 23.1982x; 23.1982x over previous
"""Trainium2 Bass kernel for an LSTM caption decoder (DecoderRNN).

Math (fp32 reference):
  x_seq = [features; embedding[captions[:, :-1]]]      # [B, T, E]
  xg    = x_seq @ W + b                                # [T*B, 4H] (t-major)
  per step t: gates = xg_t + h @ U ; i,f,o=sigmoid, g=tanh
              c = f*c + i*g ; h = o*tanh(c)
  logits = hs @ linear_w.T + linear_b                  # [B, T, V]

B=64, T=64, E=512, H=1024, V=32000.

Distribution over 8 NeuronCores: the serial recurrence (and the xg GEMM
feeding it) is replicated on every core — its PE cost is K*N-bound and
independent of batch, so batch-splitting buys nothing and collectives
would cost more than the redundant compute. The vocab projection (84% of
total FLOPs) is sharded column-wise: core c computes logits[:, :, c*4000:
(c+1)*4000]. No collectives; the host concatenates the vocab slices.

All matmuls run in float32r (TF32: full PE rate, fp32 accumulate).
Weights/activations feeding matmuls are pre-rounded to the TF32 grid.
"""

from contextlib import ExitStack

import numpy as np

import concourse.bass as bass
import concourse.mybir as mybir
import concourse.tile as tile
from concourse import bacc, bass_utils
from concourse.bass import ds

F32 = mybir.dt.float32
F32R = mybir.dt.float32r
ACTF = mybir.ActivationFunctionType
import os as _os
IMM = _os.environ.get("KIMM", "1") == "1"

B, T, E, H, V = 64, 64, 512, 1024, 32000
NCORES = 8
VS = V // NCORES          # vocab slice per core (4000)
G4 = 4 * H                # 4096
TOK = T * B               # 4096 tokens, t-major (row = t*B + b)
EC = E // 128             # 4  E chunks
HC = H // 128             # 8  H chunks
MC = TOK // 128           # 32 token chunks (2 timesteps each)
NV = 8                    # vocab N-chunks per core
VN = VS // NV             # 500 columns per vocab N-chunk


def _round_tf32(x: np.ndarray) -> np.ndarray:
    """Round fp32 to the TF32 grid (RNE to 10 mantissa bits)."""
    bits = np.ascontiguousarray(x, dtype=np.float32).view(np.uint32)
    r = bits + np.uint32(0xFFF) + ((bits >> np.uint32(13)) & np.uint32(1))
    return (r & np.uint32(0xFFFFE000)).view(np.float32)


def _emit(ctx: ExitStack, tc: tile.TileContext, io: dict, phases="abc"):
    nc = tc.nc
    xT_d, w_d, u_d, bg_d, ident_d, wt_d, bl_d = (
        io["xT"], io["w"], io["u"], io["bg"], io["ident"], io["wt"], io["bl"])
    xg_d, hsT_d, out_d = io["xg_scratch"], io["hsT_scratch"], io["logits"]
    reps = io.get("reps", 1)

    if "a" in phases:
        _phase_a(tc, nc, xT_d, w_d, bg_d, xg_d, repeat=reps)
    if "b" in phases:
        _phase_b(tc, nc, u_d, ident_d, xg_d, hsT_d, repeat=reps)
    if "c" in phases:
        _phase_c(tc, nc, wt_d, bl_d, hsT_d, out_d, repeat=reps)


def _rep_loop(tc, nc, pool, repeat):
    """Repeat-loop context for timing (reps input) or None for repeat=1."""
    if isinstance(repeat, bass.AP):
        rt = pool.tile([1, 1], mybir.dt.int32, tag="reps", name="rt")
        nc.sync.dma_start(rt[:], repeat[:, :])
        with tc.tile_critical():
            tmp = nc.alloc_registers(f"reps_regs_{nc.next_id()}")
            nc.regs_load(tmp, rt[0:1, 0:1])
            n_reps = nc.snap(tmp, donate=True, min_val=1, max_val=1024)
        return tc.For_i(0, n_reps, 1)
    return tc.For_i(0, repeat, 1) if repeat > 1 else None


def _phase_a(tc, nc, xT_d, w_d, bg_d, xg_d, repeat=1):
    # ---------------- Phase A: xg = x @ W + b  ->  DRAM [TOK, 4H] --------
    with tc.tile_pool(name="a_sb", bufs=1) as a_sb, \
         tc.tile_pool(name="a_out", bufs=3) as a_out, \
         tc.tile_pool(name="a_ps", bufs=2, space="PSUM") as a_ps:
        xT_sb = [a_sb.tile([128, TOK], F32R, tag=f"xT{k}", name=f"xT{k}") for k in range(EC)]
        w_sb = [a_sb.tile([128, G4], F32R, tag=f"w{k}", name=f"wsb{k}") for k in range(EC)]
        bg_sb = a_sb.tile([128, G4], F32, tag="bg")
        for k in range(EC):
            nc.sync.dma_start(xT_sb[k][:], xT_d[k * 128:(k + 1) * 128, :])
            nc.sync.dma_start(w_sb[k][:], w_d[k * 128:(k + 1) * 128, :])
        nc.sync.dma_start(bg_sb[:], bg_d[:, :])

        rep_cm = _rep_loop(tc, nc, a_sb, repeat)
        if rep_cm is not None:
            rep_cm.__enter__()
        for m in range(MC):
            for half in range(2):
                ps = a_ps.tile([128, 2048], F32)
                for n in range(4):
                    col = half * 2048 + n * 512
                    for k in range(EC):
                        nc.tensor.matmul(
                            ps[:, n * 512:(n + 1) * 512],
                            xT_sb[k][:, m * 128:(m + 1) * 128],
                            w_sb[k][:, col:col + 512],
                            start=(k == 0), stop=(k == EC - 1))
                o_sb = a_out.tile([128, 2048], F32R)
                nc.vector.tensor_add(
                    o_sb[:], ps[:], bg_sb[:, half * 2048:(half + 1) * 2048])
                nc.sync.dma_start(
                    xg_d[m * 128:(m + 1) * 128, half * 2048:(half + 1) * 2048],
                    o_sb[:])
        if rep_cm is not None:
            rep_cm.__exit__(None, None, None)

def _phase_b(tc, nc, u_d, ident_d, xg_d, hsT_d, repeat=1):
    # ---------------- Phase B: the recurrence ---------------------------
    # gate column ranges in xg/U: i=[0,H), f=[H,2H), g=[2H,3H), o=[3H,4H)
    # PE order i, g, f, o so the c-chain (needs i,g,f) starts earliest.
    # xg_t is injected into PSUM via an identity matmul (keeps the DVE off
    # the critical path); each gate gets its own 2-bank PSUM tile.
    with tc.tile_pool(name="b_sb", bufs=1) as b_sb, \
         tc.tile_pool(name="b_xg", bufs=2) as b_xg, \
         tc.tile_pool(name="b_gate", bufs=5) as b_gate, \
         tc.tile_pool(name="b_tmp", bufs=3) as b_tmp, \
         tc.tile_pool(name="b_hT", bufs=24) as b_hT, \
         tc.tile_pool(name="b_ps", bufs=8, space="PSUM") as b_ps:
        u_sb = [b_sb.tile([128, G4], F32R, tag=f"u{k}", name=f"usb{k}") for k in range(HC)]
        for k in range(HC):
            nc.sync.dma_start(u_sb[k][:], u_d[k * 128:(k + 1) * 128, :])
        ident = b_sb.tile([64, 64], F32R, tag="ident")
        nc.sync.dma_start(ident[:], ident_d[:, :])
        c_st = b_sb.tile([64, H], F32, tag="c")     # persistent cell state
        h_st = b_sb.tile([64, H], F32, tag="h")     # persistent hidden (pre-T)

        rep_cm = _rep_loop(tc, nc, b_sb, repeat)
        if rep_cm is not None:
            rep_cm.__enter__()
        hT_prev = None
        for t in range(T):
            # xg_t [64, 4H] in one contiguous DMA (f32r, bias folded in)
            xg_t = b_xg.tile([64, G4], F32R, tag="xg")
            nc.sync.dma_start(xg_t[:], xg_d[t * 64:(t + 1) * 64, :])
            xg_g = [xg_t[:, g * H:(g + 1) * H] for g in range(4)]

            # One 1-bank PSUM tile per (gate, half). The I-MMs (psum = xg)
            # depend only on the xg DMA, so the PE can run them during the
            # previous step's elementwise tail. Emit them in pairs right
            # before their gate's U-MMs to avoid PE head-of-line blocking
            # on late PSUM slot recycling.
            GORDER = (0, 2, 1, 3)          # i, g, f, o
            ps_gh = {}

            def imm(g):
                for n2 in range(2):
                    ps = b_ps.tile([64, 512], F32, tag="ps",
                                   name=f"ps{g}_{n2}_{t}")
                    nc.tensor.matmul(
                        ps[:], ident[:], xg_g[g][:, n2 * 512:(n2 + 1) * 512],
                        start=True, stop=(t == 0))
                    ps_gh[(g, n2)] = ps

            def umms(g):
                for n2 in range(2):
                    ucol = g * H + n2 * 512
                    for k in range(HC):
                        nc.tensor.matmul(
                            ps_gh[(g, n2)][:], hT_prev[k][:],
                            u_sb[k][:, ucol:ucol + 512],
                            start=False, stop=(k == HC - 1))

            if t == 0:
                for g in GORDER:
                    imm(g)
            else:
                imm(0), imm(2)
                umms(0), umms(2)
                imm(1), imm(3)
                umms(1), umms(3)

            def act(g):
                a = b_gate.tile([64, H], F32, tag="gate", name=f"gate{g}_{t}")
                func = ACTF.Tanh if g == 2 else ACTF.Sigmoid
                for n2 in range(2):
                    nc.scalar.activation(a[:, n2 * 512:(n2 + 1) * 512],
                                         ps_gh[(g, n2)][:], func)
                return a

            # elementwise chain in 512-halves: i,g first, then f, then o
            i_t = act(0)
            g_t = act(2)
            ig = b_tmp.tile([64, H], F32, tag="tmp", name=f"ig{t}")
            f_t = act(1)
            tc_t = b_tmp.tile([64, H], F32, tag="tmp", name=f"tc{t}")
            o_t = act(3)
            hh_done = []
            for hh in range(2):
                sl = slice(hh * 512, (hh + 1) * 512)
                nc.vector.tensor_mul(ig[:, sl], i_t[:, sl], g_t[:, sl])
                if t == 0:
                    nc.vector.tensor_copy(c_st[:, sl], ig[:, sl])
                else:
                    nc.vector.tensor_mul(c_st[:, sl], f_t[:, sl], c_st[:, sl])
                    nc.vector.tensor_add(c_st[:, sl], c_st[:, sl], ig[:, sl])
                nc.scalar.activation(tc_t[:, sl], c_st[:, sl], ACTF.Tanh)
                nc.vector.tensor_mul(h_st[:, sl], o_t[:, sl], tc_t[:, sl])
                # transpose this half's 4 chunks right away
                for k in range(hh * 4, hh * 4 + 4):
                    tp = b_ps.tile([128, 64], F32, tag="ps")
                    nc.tensor.transpose(tp[:], h_st[:, k * 128:(k + 1) * 128],
                                        ident[:].bitcast(F32))
                    ht_k = b_hT.tile([128, 64], F32R, tag="hT",
                                     name=f"hT{k}_{t}")
                    nc.vector.tensor_copy(ht_k[:], tp[:])
                    nc.scalar.dma_start(
                        hsT_d[:, t * 512 + k * 64:t * 512 + (k + 1) * 64],
                        ht_k[:])
                    hh_done.append(ht_k)
            hT_prev = hh_done
        if rep_cm is not None:
            rep_cm.__exit__(None, None, None)

def _phase_c(tc, nc, wt_d, bl_d, hsT_d, out_d, repeat=1):
    # ---------------- Phase C: logits slice = hs @ WT + bl ---------------
    with tc.tile_pool(name="c_sb", bufs=1) as c_sb, \
         tc.tile_pool(name="c_hs", bufs=24) as c_hs, \
         tc.tile_pool(name="c_out", bufs=12) as c_out, \
         tc.tile_pool(name="c_ps", bufs=8, space="PSUM") as c_ps:
        wt_sb = [c_sb.tile([128, VS], F32R, tag=f"wt{k}", name=f"wtsb{k}") for k in range(HC)]
        for k in range(HC):
            nc.sync.dma_start(wt_sb[k][:], wt_d[k * 128:(k + 1) * 128, :])
        bl_sb = c_sb.tile([128, VS], F32, tag="bl")
        nc.sync.dma_start(bl_sb[:], bl_d[:, :])

        hsT4 = hsT_d.rearrange("p (t k b) -> p t k b", t=T, k=HC, b=64)
        rep_cm = _rep_loop(tc, nc, c_sb, repeat)
        if rep_cm is not None:
            rep_cm.__enter__()
        for m in range(MC):
            t0, t1 = 2 * m, 2 * m + 1
            hs_k = []
            for k in range(HC):
                hk = c_hs.tile([128, 128], F32R, tag="slab", name=f"hk{k}_{m}")
                nc.sync.dma_start(
                    hk[:].rearrange("p (t b) -> p t b", t=2, b=64),
                    hsT4[:, t0:t0 + 2, k, :])
                hs_k.append(hk)
            for n in range(NV):
                ps = c_ps.tile([128, VN], F32)
                for k in range(HC):
                    nc.tensor.matmul(
                        ps[:], hs_k[k][:],
                        wt_sb[k][:, n * VN:(n + 1) * VN],
                        start=(k == 0), stop=(k == HC - 1))
                o_sb = c_out.tile([128, VN], F32)
                nc.vector.tensor_add(o_sb[:], ps[:],
                                     bl_sb[:, n * VN:(n + 1) * VN])
                eng0 = (nc.sync, nc.scalar)[n % 2]
                eng1 = (nc.scalar, nc.sync)[n % 2]
                eng0.dma_start(out_d[:, t0, ds(n * VN, VN)], o_sb[0:64, :])
                eng1.dma_start(out_d[:, t1, ds(n * VN, VN)], o_sb[64:128, :])
        if rep_cm is not None:
            rep_cm.__exit__(None, None, None)


def build_program(phases=None, with_reps=False):
    import os
    if phases is None:
        phases = os.environ.get("KPHASES", "abc")
    nc = bacc.Bacc("TRN2", target_bir_lowering=False, debug=False,
                   num_devices=NCORES)
    io = {}
    if with_reps:
        io["reps"] = nc.dram_tensor("reps", [1, 1], mybir.dt.int32,
                                    kind="ExternalInput").ap()
    io |= {
        "xT": nc.dram_tensor("xT", [E, TOK], F32R, kind="ExternalInput").ap(),
        "w": nc.dram_tensor("w", [E, G4], F32R, kind="ExternalInput").ap(),
        "u": nc.dram_tensor("u", [H, G4], F32R, kind="ExternalInput").ap(),
        "bg": nc.dram_tensor("bg", [128, G4], F32, kind="ExternalInput").ap(),
        "ident": nc.dram_tensor("ident", [64, 64], F32R,
                                kind="ExternalInput").ap(),
        "wt": nc.dram_tensor("wt", [H, VS], F32R, kind="ExternalInput").ap(),
        "bl": nc.dram_tensor("bl", [128, VS], F32, kind="ExternalInput").ap(),
        "xg_scratch": nc.dram_tensor("xg_scratch", [TOK, G4], F32R,
                                     kind="Internal").ap(),
        "hsT_scratch": nc.dram_tensor("hsT_scratch", [128, T * 512], F32R,
                                      kind="Internal").ap(),
        "logits": nc.dram_tensor("logits", [B, T, VS], F32,
                                 kind="ExternalOutput").ap(),
    }
    with tile.TileContext(nc) as tc:
        with ExitStack() as ctx:
            _emit(ctx, tc, io, phases)
    nc.compile()
    return nc


def make_in_maps(features, captions, embedding, W_i, U_i, b_i, W_f, U_f, b_f,
                 W_g, U_g, b_g, W_o, U_o, b_o, linear_w, linear_b):
    features = np.asarray(features, dtype=np.float32)
    captions = np.asarray(captions)
    embedding = np.asarray(embedding, dtype=np.float32)
    emb = embedding[captions[:, :-1]]                        # [B, T-1, E]
    x_seq = np.concatenate([features[:, None, :], emb], axis=1)  # [B, T, E]
    x_flat = np.ascontiguousarray(
        x_seq.transpose(1, 0, 2).reshape(TOK, E))            # t-major tokens
    xT = _round_tf32(np.ascontiguousarray(x_flat.T))         # [E, TOK]

    w = _round_tf32(np.concatenate([W_i, W_f, W_g, W_o], axis=1))  # [E, 4H]
    u = _round_tf32(np.concatenate([U_i, U_f, U_g, U_o], axis=1))  # [H, 4H]
    bgv = np.concatenate([b_i, b_f, b_g, b_o], axis=0).astype(np.float32)
    bg = np.ascontiguousarray(np.broadcast_to(bgv[None, :], (128, G4)))
    ident = np.eye(64, dtype=np.float32)

    linear_w = np.asarray(linear_w, dtype=np.float32)
    linear_b = np.asarray(linear_b, dtype=np.float32)
    common = {"xT": xT, "w": w, "u": u, "bg": bg, "ident": ident}
    in_maps = []
    for c in range(NCORES):
        wt = _round_tf32(
            np.ascontiguousarray(linear_w[c * VS:(c + 1) * VS, :].T))
        bl = np.ascontiguousarray(np.broadcast_to(
            linear_b[None, c * VS:(c + 1) * VS], (128, VS)))
        in_maps.append({**common, "wt": wt, "bl": bl})
    return in_maps


_PROGRAM = None


def kernel(**inputs) -> np.ndarray:
    global _PROGRAM
    if _PROGRAM is None:
        _PROGRAM = build_program()
    in_maps = make_in_maps(**inputs)
    res = bass_utils.run_bass_kernel_spmd(
        _PROGRAM, in_maps, core_ids=list(range(NCORES)))
    out = np.empty((B, T, V), dtype=np.float32)
    for c in range(NCORES):
        out[:, :, c * VS:(c + 1) * VS] = res.results[c]["logits"]
    return out
